# revision 7
# baseline (speedup 1.0000x reference)
"""Trainium2 Bass kernel for ArgumentGCN-with-reverse-edges (double relations).

Math (per batch b, N=256 nodes, D=768):
  dd     = mask outer product, zero diag (symmetric 0/1)
  g_a    = dd*A, g_p = dd*P, g_ar = dd*A^T, g_pr = dd*P^T   (dd idempotent)
  dw     = sigmoid(node @ Wq^T + bq)
  self   = node @ Ws^T + bs
  agg    = sum_x (dw-col-scaled g_x) @ (node @ Wxc^T),  Wxc = Wx + Wx2 (fused)
  out    = relu(self + agg / max(neigh,1)),  neigh = row-sums of the 4 graphs

Device mapping (per core: 8 batches, pure data parallel over 8 cores):
  - One fused "sweep" matmul per batch: nodeT[768,256] x Wcat[768,3841]
    where Wcat = [Ws^T | Wac^T | Wpc^T | Wapc^T | Wppc^T | Wq^T] (bf16),
    K=768 accumulated in PSUM f32, 384-wide output chunks (1 bank each).
  - Adjacency matmul K=1024: stacked dw-scaled graphs (lhsT, no device
    transposes needed: host passes A and A^T; dd symmetry gives the rest)
    against the 4 projections from the sweep.
  - neigh via matmul with the bf16 mask vector (exact integer arithmetic).
  - Mask handled honestly: dwm = dw*m scales lhsT rows; the m_i row factor
    folds into rdenom = m / max(neigh,1).
  - Two-stage software pipeline: PE does sweep(b) then adjacency(b-1), so
    the DVE/ACT drain + lhsT-construction latency of batch b never blocks
    the tensor engine.
"""

import os
import numpy as np
import ml_dtypes

BF = ml_dtypes.bfloat16

B, N, D = 64, 256, 768
NCORES = 8
NB = B // NCORES          # batches per core
CW = 384                  # sweep psum chunk width
SWEEP_COLS = 5 * D + 1    # 3841
CHUNK_ORDER = [9] + list(range(9))  # dw chunk first

_cached = {}


def _build_nc():
    import concourse.tile as tile
    from concourse import bacc, mybir
    from contextlib import ExitStack

    BF16 = mybir.dt.bfloat16
    F32 = mybir.dt.float32
    AF = mybir.ActivationFunctionType
    ALU = mybir.AluOpType

    nc = bacc.Bacc(
        "TRN2",
        target_bir_lowering=False,
        debug=False,
        enable_asserts=False,
        num_devices=NCORES,
    )

    nodeT = nc.dram_tensor("nodeT", [NB, D, N], BF16, kind="ExternalInput")
    gA_d = nc.dram_tensor("ga", [NB, N, N], BF16, kind="ExternalInput")
    gAT_d = nc.dram_tensor("gat", [NB, N, N], BF16, kind="ExternalInput")
    gP_d = nc.dram_tensor("gp", [NB, N, N], BF16, kind="ExternalInput")
    gPT_d = nc.dram_tensor("gpt", [NB, N, N], BF16, kind="ExternalInput")
    mask_d = nc.dram_tensor("mask", [NB, N], F32, kind="ExternalInput")
    wcat_d = nc.dram_tensor("wcat", [D, SWEEP_COLS], BF16, kind="ExternalInput")
    om_d = nc.dram_tensor("om", [N, N], BF16, kind="ExternalInput")
    bsb_d = nc.dram_tensor("bsb", [128, D], F32, kind="ExternalInput")
    bqb_d = nc.dram_tensor("bqb", [128, 1], F32, kind="ExternalInput")
    out_d = nc.dram_tensor("out", [NB, N, D], F32, kind="ExternalOutput")
    aw_d = nc.dram_tensor("aw", [NB, N], F32, kind="ExternalOutput")

    with tile.TileContext(nc) as tc, ExitStack() as ctx:
        const = ctx.enter_context(tc.tile_pool(name="const", bufs=1))
        inp = ctx.enter_context(tc.tile_pool(name="inp", bufs=3))
        work = ctx.enter_context(tc.tile_pool(name="work", bufs=2))
        outp = ctx.enter_context(tc.tile_pool(name="outp", bufs=3))
        ps_swp = ctx.enter_context(tc.tile_pool(name="ps_swp", bufs=4, space="PSUM"))
        ps_adj = ctx.enter_context(tc.tile_pool(name="ps_adj", bufs=3, space="PSUM"))
        ps_ng = ctx.enter_context(tc.tile_pool(name="ps_ng", bufs=1, space="PSUM"))

        # weights DMA'd per column-chunk so the first sweep can start early.
        # Startup-critical DMAs are issued first and spread over three DMA
        # queues (sync/scalar HWDGE + gpsimd) — a single queue serializes.
        wcat = const.tile([128, 6, SWEEP_COLS], BF16)
        wcat_r = wcat_d.rearrange("(c p) n -> p c n", p=128)
        om = const.tile([128, 2, N], BF16)
        bsb = const.tile([128, D], F32)
        bqb = const.tile([128, 1], F32)

        def load_consts_tail():
            nc.sync.dma_start(om[:], om_d.rearrange("(t p) n -> p t n", p=128))
            nc.sync.dma_start(bsb[:], bsb_d[:])
            nc.sync.dma_start(bqb[:], bqb_d[:])

        def load_wcat_chunk(c, eng):
            w = 385 if c == 9 else 384
            eng.dma_start(
                wcat[:, :, c * CW : c * CW + w], wcat_r[:, :, c * CW : c * CW + w]
            )

        def sweep(b):
            """DMA inputs; fused sweep matmul; drains; dw sigmoid; prep of
            the adjacency lhsT stack (DVE/GpSimd, overlaps next PE work)."""
            t = {}
            nT = inp.tile([128, 6, N], BF16, name="nT", tag="nT")
            nc.sync.dma_start(nT[:], nodeT[b].rearrange("(c p) n -> p c n", p=128))
            if b == 0:
                # weight chunks follow nT, alternating between the sync and
                # scalar HWDGE queues so they land ahead of the matmuls.
                for i, c in enumerate(CHUNK_ORDER):
                    load_wcat_chunk(c, nc.sync if i % 2 == 0 else nc.scalar)
            geng = nc.sync
            t["gA"] = gA = inp.tile([128, 2, N], BF16, name="gA", tag="gA")
            geng.dma_start(gA[:], gA_d[b].rearrange("(t p) n -> p t n", p=128))
            t["gAT"] = gAT = inp.tile([128, 2, N], BF16, name="gAT", tag="gAT")
            geng.dma_start(gAT[:], gAT_d[b].rearrange("(t p) n -> p t n", p=128))
            t["gP"] = gP = inp.tile([128, 2, N], BF16, name="gP", tag="gP")
            geng.dma_start(gP[:], gP_d[b].rearrange("(t p) n -> p t n", p=128))
            t["gPT"] = gPT = inp.tile([128, 2, N], BF16, name="gPT", tag="gPT")
            geng.dma_start(gPT[:], gPT_d[b].rearrange("(t p) n -> p t n", p=128))
            t["m32"] = m32 = inp.tile([128, 2], F32, name="m32", tag="m32")
            geng.dma_start(m32[:], mask_d[b].rearrange("(t p) -> p t", p=128))
            if b == 0:
                load_consts_tail()

            t["dw"] = dw = work.tile([128, 2], F32, name="dw", tag="dw")
            t["selfi"] = selfi = work.tile([128, 2, D], F32, name="selfi", tag="selfi")
            t["proj"] = proj = work.tile([128, 8, D], BF16, name="proj", tag="proj")

            for mt in range(2):
                lhs = nT[:, :, mt * 128 : (mt + 1) * 128]
                for c in CHUNK_ORDER:
                    w = 385 if c == 9 else 384
                    ps = ps_swp.tile([128, 512], F32, name="swp", tag="swp")
                    for k in range(6):
                        nc.tensor.matmul(
                            ps[:, :w],
                            lhs[:, k, :],
                            wcat[:, k, c * CW : c * CW + w],
                            start=(k == 0),
                            stop=(k == 5),
                        )
                    if c < 2:
                        csl = slice(c * CW, (c + 1) * CW)
                        nc.vector.tensor_add(selfi[:, mt, csl], ps[:, :CW], bsb[:, csl])
                    else:
                        r, h = divmod(c - 2, 2)
                        dst = proj[:, r * 2 + mt, h * CW : (h + 1) * CW]
                        if r % 2 == 0:
                            nc.vector.tensor_copy(dst, ps[:, :CW])
                        else:
                            nc.scalar.activation(dst, ps[:, :CW], AF.Copy)
                        if c == 9:
                            nc.scalar.activation(
                                dw[:, mt : mt + 1], ps[:, 384:385], AF.Sigmoid,
                                bias=bqb[:],
                            )

            nc.sync.dma_start(aw_d[b].rearrange("(t p) -> p t", p=128), dw[:])

            # ---- adjacency prep: DVE small ops + GpSimd bulk elementwise ----
            t["m16"] = m16 = work.tile([128, 2], BF16, name="m16", tag="m16")
            nc.vector.tensor_copy(m16[:], m32[:])
            dwm = work.tile([128, 2], F32, name="dwm", tag="dwm")
            nc.vector.tensor_mul(dwm[:], dw[:], m32[:])
            ddw = work.tile([128, 2, N], BF16, name="ddw", tag="ddw")
            for tt in range(2):
                nc.vector.tensor_scalar_mul(ddw[:, tt, :], om[:, tt, :], dwm[:, tt : tt + 1])
            t["adjL"] = adjL = work.tile([128, 4, 2, N], BF16, name="adjL", tag="adjL")
            for r, g in enumerate([gAT, gPT, gA, gP]):
                nc.gpsimd.tensor_tensor(adjL[:, r, :, :], g[:], ddw[:], op=ALU.mult)
            t["ssm"] = ssm = work.tile([128, 2, N], BF16, name="ssm", tag="ssm")
            s2 = work.tile([128, 2, N], BF16, name="s2", tag="s2")
            nc.gpsimd.tensor_add(ssm[:], gA[:], gAT[:])
            nc.gpsimd.tensor_add(s2[:], gP[:], gPT[:])
            nc.gpsimd.tensor_add(ssm[:], ssm[:], s2[:])
            nc.gpsimd.tensor_tensor(ssm[:], ssm[:], om[:], op=ALU.mult)
            return t

        def finish(b, t):
            """neigh + rdenom; adjacency matmuls; fused epilogue; out DMA."""
            png = ps_ng.tile([128, 2], F32, name="ng", tag="ng")
            for ti in range(2):
                for kc in range(2):
                    nc.tensor.matmul(
                        png[:, ti : ti + 1],
                        t["ssm"][:, kc, ti * 128 : (ti + 1) * 128],
                        t["m16"][:, kc : kc + 1],
                        start=(kc == 0),
                        stop=(kc == 1),
                    )
            nn = work.tile([128, 2], F32, name="nn", tag="nn")
            rd = work.tile([128, 2], F32, name="rd", tag="rd")
            nc.vector.tensor_mul(nn[:], png[:], t["m32"][:])
            nc.vector.tensor_scalar_max(nn[:], nn[:], 1.0)
            nc.vector.reciprocal(nn[:], nn[:])
            nc.vector.tensor_mul(rd[:], nn[:], t["m32"][:])

            adjL, proj, selfi = t["adjL"], t["proj"], t["selfi"]
            for mt in range(2):
                outt = outp.tile([128, D], F32, name="outt", tag="outt")
                outr = outp.tile([128, D], F32, name="outr", tag="outr")
                for h in range(2):
                    hsl = slice(h * CW, (h + 1) * CW)
                    pa = ps_adj.tile([128, CW], F32, name="adj", tag="adj")
                    for kc in range(8):
                        r, tt = divmod(kc, 2)
                        nc.tensor.matmul(
                            pa[:],
                            adjL[:, r, tt, mt * 128 : (mt + 1) * 128],
                            proj[:, kc, hsl],
                            start=(kc == 0),
                            stop=(kc == 7),
                        )
                    nc.vector.scalar_tensor_tensor(
                        outt[:, hsl], pa[:], rd[:, mt : mt + 1], selfi[:, mt, hsl],
                        op0=ALU.mult, op1=ALU.add,
                    )
                nc.scalar.activation(outr[:], outt[:], AF.Relu)
                nc.sync.dma_start(out_d[b, mt * 128 : (mt + 1) * 128, :], outr[:])

        prev = None
        for b in range(NB):
            t = sweep(b)
            if prev is not None:
                finish(b - 1, prev)
            prev = t
        finish(NB - 1, prev)

    nc.compile()
    return nc


def _get_nc():
    if "nc" not in _cached:
        _cached["nc"] = _build_nc()
    return _cached["nc"]


def _prep_in_maps(node, node_mask, argument_graph, punctuation_graph,
                  Wq, bq, Ws, bs, Wa, Wp, Wap, Wpp, Wa2, Wp2, Wap2, Wpp2):
    f32 = np.float32
    node = np.asarray(node, f32)
    mf = np.asarray(node_mask).astype(f32)
    A = np.asarray(argument_graph)
    P = np.asarray(punctuation_graph)

    Wac = np.asarray(Wa, f32) + np.asarray(Wa2, f32)
    Wpc = np.asarray(Wp, f32) + np.asarray(Wp2, f32)
    Wapc = np.asarray(Wap, f32) + np.asarray(Wap2, f32)
    Wppc = np.asarray(Wpp, f32) + np.asarray(Wpp2, f32)
    wcat = np.concatenate(
        [np.asarray(Ws, f32).T, Wac.T, Wpc.T, Wapc.T, Wppc.T, np.asarray(Wq, f32).T],
        axis=1,
    ).astype(BF)                       # [768, 3841]
    om = (1.0 - np.eye(N, dtype=f32)).astype(BF)
    bsb = np.broadcast_to(np.asarray(bs, f32), (128, D)).copy()
    bqb = np.full((128, 1), float(np.asarray(bq).reshape(-1)[0]), f32)

    nodeT = np.ascontiguousarray(node.transpose(0, 2, 1)).astype(BF)  # [B, D, N]
    ga = A.astype(BF)
    gat = np.ascontiguousarray(np.swapaxes(A, 1, 2)).astype(BF)
    gp = P.astype(BF)
    gpt = np.ascontiguousarray(np.swapaxes(P, 1, 2)).astype(BF)

    in_maps = []
    for c in range(NCORES):
        sl = slice(c * NB, (c + 1) * NB)
        in_maps.append(
            dict(
                nodeT=nodeT[sl], ga=ga[sl], gat=gat[sl], gp=gp[sl], gpt=gpt[sl],
                mask=mf[sl], wcat=wcat, om=om, bsb=bsb, bqb=bqb,
            )
        )
    return in_maps


def _run(inputs, trace=False):
    from concourse.bass_utils import run_bass_kernel_spmd

    nc = _get_nc()
    in_maps = _prep_in_maps(**inputs)
    res = run_bass_kernel_spmd(
        nc, in_maps, core_ids=list(range(NCORES)), trace=trace
    )
    node_out = np.concatenate(
        [np.asarray(res.results[c]["out"], np.float32) for c in range(NCORES)], axis=0
    )
    aw = np.concatenate(
        [np.asarray(res.results[c]["aw"], np.float32) for c in range(NCORES)], axis=0
    )
    return (node_out, aw[:, None, :]), res


def kernel(**inputs):
    out, _ = _run(inputs, trace=False)
    return out


# revision 9
# speedup vs baseline: 1.4470x; 1.4470x over previous
"""Trainium2 Bass kernel for ArgumentGCN-with-reverse-edges (double relations).

Math (per batch b, N=256 nodes, D=768):
  dd     = mask outer product, zero diag (symmetric 0/1)
  g_a    = dd*A, g_p = dd*P, g_ar = dd*A^T, g_pr = dd*P^T   (dd idempotent)
  dw     = sigmoid(node @ Wq^T + bq)
  self   = node @ Ws^T + bs
  agg    = sum_x (dw-col-scaled g_x) @ (node @ Wxc^T),  Wxc = Wx + Wx2 (fused)
  out    = relu(self + agg / max(neigh,1)),  neigh = row-sums of the 4 graphs

Device mapping (per core: 8 batches, pure data parallel over 8 cores):
  - self_info + dw score: bf16 sweep  nodeT[768,256] x [Ws^T | Wq^T].
  - 4 relation projections: fp8(e4m3) DoubleRow matmuls — node and the
    combined relation weights (scaled by WSCALE to fit e4m3 range) packed
    as [128, kc, 2, *] K-pairs.  agg is only ~2% of output magnitude, so
    fp8 there is invisible at the output (verified ~3e-3 rel err).
  - Adjacency matmul K=1024 in fp8 DoubleRow: stacked dw-scaled graphs
    (no device transposes: host passes A and A^T; dd symmetry gives the
    rest) against the fp8 projections.
  - neigh via bf16 matmul with the mask vector (exact integer arithmetic).
  - Mask handled honestly: dwm = dw*m scales lhsT rows; the m_i row factor
    and the 1/WSCALE both fold into rdenom = m / (WSCALE * max(neigh,1)).
  - Two-stage software pipeline: PE does sweep(b) then adjacency(b-1), so
    drain/lhsT-construction latency never blocks the tensor engine.
"""

import os
import numpy as np
import ml_dtypes

BF = ml_dtypes.bfloat16
F8 = ml_dtypes.float8_e4m3

B, N, D = 64, 256, 768
NCORES = 8
NB = B // NCORES          # batches per core
CW = 384                  # psum chunk width
WSCALE = 32.0             # relation-weight scale for e4m3 range

_cached = {}


def _build_nc():
    import concourse.tile as tile
    from concourse import bacc, mybir
    from contextlib import ExitStack

    BF16 = mybir.dt.bfloat16
    FP8 = mybir.dt.float8e4
    F32 = mybir.dt.float32
    AF = mybir.ActivationFunctionType
    ALU = mybir.AluOpType
    DR = mybir.MatmulPerfMode.DoubleRow

    nc = bacc.Bacc(
        "TRN2",
        target_bir_lowering=False,
        debug=False,
        enable_asserts=False,
        num_devices=NCORES,
    )

    nodeT = nc.dram_tensor("nodeT", [NB, D, N], BF16, kind="ExternalInput")
    node8_d = nc.dram_tensor("node8", [NB, 128, 3, 2, N], FP8, kind="ExternalInput")
    gA_d = nc.dram_tensor("ga", [NB, N, N], BF16, kind="ExternalInput")
    gAT_d = nc.dram_tensor("gat", [NB, N, N], BF16, kind="ExternalInput")
    gP_d = nc.dram_tensor("gp", [NB, N, N], BF16, kind="ExternalInput")
    gPT_d = nc.dram_tensor("gpt", [NB, N, N], BF16, kind="ExternalInput")
    mask_d = nc.dram_tensor("mask", [NB, N], F32, kind="ExternalInput")
    wbf_d = nc.dram_tensor("wbf", [D, D + 1], BF16, kind="ExternalInput")
    w8_d = nc.dram_tensor("w8", [128, 3, 2, 4 * D], FP8, kind="ExternalInput")
    om_d = nc.dram_tensor("om", [N, N], BF16, kind="ExternalInput")
    bsb_d = nc.dram_tensor("bsb", [128, D], F32, kind="ExternalInput")
    bqb_d = nc.dram_tensor("bqb", [128, 1], F32, kind="ExternalInput")
    out_d = nc.dram_tensor("out", [NB, N, D], F32, kind="ExternalOutput")
    aw_d = nc.dram_tensor("aw", [NB, N], F32, kind="ExternalOutput")

    with tile.TileContext(nc) as tc, ExitStack() as ctx:
        const = ctx.enter_context(tc.tile_pool(name="const", bufs=1))
        inp = ctx.enter_context(tc.tile_pool(name="inp", bufs=3))
        work = ctx.enter_context(tc.tile_pool(name="work", bufs=2))
        outp = ctx.enter_context(tc.tile_pool(name="outp", bufs=3))
        ps_swp = ctx.enter_context(tc.tile_pool(name="ps_swp", bufs=4, space="PSUM"))
        ps_adj = ctx.enter_context(tc.tile_pool(name="ps_adj", bufs=3, space="PSUM"))
        ps_ng = ctx.enter_context(tc.tile_pool(name="ps_ng", bufs=1, space="PSUM"))

        wbf = const.tile([128, 6, D + 1], BF16, name="wbf")
        w8 = const.tile([128, 3, 2, 4 * D], FP8, name="w8")
        om = const.tile([128, 2, N], BF16, name="om")
        bsb = const.tile([128, D], F32, name="bsb")
        bqb = const.tile([128, 1], F32, name="bqb")
        wbf_r = wbf_d.rearrange("(c p) n -> p c n", p=128)

        def sweep(b):
            """DMA inputs; bf16 self/dw sweep; fp8 relation projections;
            adjacency-prep on DVE/GpSimd (overlaps next PE work)."""
            t = {}
            nT = inp.tile([128, 6, N], BF16, name="nT", tag="nT")
            nc.sync.dma_start(nT[:], nodeT[b].rearrange("(c p) n -> p c n", p=128))
            if b == 0:
                # startup-critical loads in consumption order on one queue
                nc.sync.dma_start(wbf[:, :, CW:], wbf_r[:, :, CW:])
                nc.sync.dma_start(wbf[:, :, :CW], wbf_r[:, :, :CW])
            n8 = inp.tile([128, 3, 2, N], FP8, name="n8", tag="n8")
            nc.sync.dma_start(n8[:], node8_d[b])
            if b == 0:
                for c in range(8):
                    csl = slice(c * CW, (c + 1) * CW)
                    nc.sync.dma_start(w8[:, :, :, csl], w8_d[:, :, :, csl])
            t["gA"] = gA = inp.tile([128, 2, N], BF16, name="gA", tag="gA")
            nc.sync.dma_start(gA[:], gA_d[b].rearrange("(t p) n -> p t n", p=128))
            t["gAT"] = gAT = inp.tile([128, 2, N], BF16, name="gAT", tag="gAT")
            nc.sync.dma_start(gAT[:], gAT_d[b].rearrange("(t p) n -> p t n", p=128))
            t["gP"] = gP = inp.tile([128, 2, N], BF16, name="gP", tag="gP")
            nc.sync.dma_start(gP[:], gP_d[b].rearrange("(t p) n -> p t n", p=128))
            t["gPT"] = gPT = inp.tile([128, 2, N], BF16, name="gPT", tag="gPT")
            nc.sync.dma_start(gPT[:], gPT_d[b].rearrange("(t p) n -> p t n", p=128))
            t["m32"] = m32 = inp.tile([128, 2], F32, name="m32", tag="m32")
            nc.sync.dma_start(m32[:], mask_d[b].rearrange("(t p) -> p t", p=128))
            if b == 0:
                nc.sync.dma_start(om[:], om_d.rearrange("(t p) n -> p t n", p=128))
                nc.sync.dma_start(bsb[:], bsb_d[:])
                nc.sync.dma_start(bqb[:], bqb_d[:])

            t["dw"] = dw = work.tile([128, 2], F32, name="dw", tag="dw")
            t["selfi"] = selfi = work.tile([128, 2, D], F32, name="selfi", tag="selfi")
            t["proj"] = proj = work.tile([128, 8, D], FP8, name="proj", tag="proj")

            for mt in range(2):
                lhs = nT[:, :, mt * 128 : (mt + 1) * 128]
                # bf16: [Ws cols 384:768 | wq] first so dw is ready early
                ps1 = ps_swp.tile([128, 512], F32, name="swp", tag="swp")
                for k in range(6):
                    nc.tensor.matmul(
                        ps1[:, : CW + 1], lhs[:, k, :], wbf[:, k, CW:],
                        start=(k == 0), stop=(k == 5),
                    )
                nc.vector.tensor_add(selfi[:, mt, CW:], ps1[:, :CW], bsb[:, CW:])
                nc.scalar.activation(
                    dw[:, mt : mt + 1], ps1[:, CW : CW + 1], AF.Sigmoid, bias=bqb[:]
                )
                ps0 = ps_swp.tile([128, 512], F32, name="swp", tag="swp")
                for k in range(6):
                    nc.tensor.matmul(
                        ps0[:, :CW], lhs[:, k, :], wbf[:, k, :CW],
                        start=(k == 0), stop=(k == 5),
                    )
                nc.vector.tensor_add(selfi[:, mt, :CW], ps0[:, :CW], bsb[:, :CW])

                # fp8 DoubleRow relation projections
                lhs8 = n8[:, :, :, mt * 128 : (mt + 1) * 128]
                for c in range(8):
                    ps = ps_swp.tile([128, 512], F32, name="swp", tag="swp")
                    csl = slice(c * CW, (c + 1) * CW)
                    for kc in range(3):
                        nc.tensor.matmul(
                            ps[:, :CW], lhs8[:, kc, :, :], w8[:, kc, :, csl],
                            start=(kc == 0), stop=(kc == 2), perf_mode=DR,
                        )
                    r, h = divmod(c, 2)
                    dst = proj[:, r * 2 + mt, h * CW : (h + 1) * CW]
                    if c % 2 == 0:
                        nc.vector.tensor_copy(dst, ps[:, :CW])
                    else:
                        nc.scalar.activation(dst, ps[:, :CW], AF.Copy)

            nc.sync.dma_start(aw_d[b].rearrange("(t p) -> p t", p=128), dw[:])

            # ---- adjacency prep: DVE small ops + GpSimd bulk elementwise ----
            t["m16"] = m16 = work.tile([128, 2], BF16, name="m16", tag="m16")
            nc.vector.tensor_copy(m16[:], m32[:])
            dwm = work.tile([128, 2], F32, name="dwm", tag="dwm")
            nc.vector.tensor_mul(dwm[:], dw[:], m32[:])
            ddw = work.tile([128, 2, N], BF16, name="ddw", tag="ddw")
            for tt in range(2):
                nc.vector.tensor_scalar_mul(ddw[:, tt, :], om[:, tt, :], dwm[:, tt : tt + 1])
            t["adjL"] = adjL = work.tile([128, 4, 2, N], FP8, name="adjL", tag="adjL")
            for r, g in enumerate([gAT, gPT, gA, gP]):
                nc.gpsimd.tensor_tensor(adjL[:, r, :, :], g[:], ddw[:], op=ALU.mult)
            t["ssm"] = ssm = work.tile([128, 2, N], BF16, name="ssm", tag="ssm")
            s2 = work.tile([128, 2, N], BF16, name="s2", tag="s2")
            nc.gpsimd.tensor_add(ssm[:], gA[:], gAT[:])
            nc.gpsimd.tensor_add(s2[:], gP[:], gPT[:])
            nc.gpsimd.tensor_add(ssm[:], ssm[:], s2[:])
            nc.gpsimd.tensor_tensor(ssm[:], ssm[:], om[:], op=ALU.mult)
            return t

        def finish(b, t):
            """neigh + rdenom; fp8 adjacency matmuls; fused epilogue; out."""
            png = ps_ng.tile([128, 2], F32, name="ng", tag="ng")
            for ti in range(2):
                for kc in range(2):
                    nc.tensor.matmul(
                        png[:, ti : ti + 1],
                        t["ssm"][:, kc, ti * 128 : (ti + 1) * 128],
                        t["m16"][:, kc : kc + 1],
                        start=(kc == 0), stop=(kc == 1),
                    )
            nn = work.tile([128, 2], F32, name="nn", tag="nn")
            rd = work.tile([128, 2], F32, name="rd", tag="rd")
            nc.vector.tensor_mul(nn[:], png[:], t["m32"][:])
            nc.vector.tensor_scalar_max(nn[:], nn[:], 1.0)
            nc.vector.reciprocal(nn[:], nn[:])
            # rd = m / (WSCALE * max(neigh,1))
            nc.vector.scalar_tensor_tensor(
                rd[:], nn[:], 1.0 / WSCALE, t["m32"][:], op0=ALU.mult, op1=ALU.mult
            )

            adjL, proj, selfi = t["adjL"], t["proj"], t["selfi"]
            for mt in range(2):
                outt = outp.tile([128, D], F32, name="outt", tag="outt")
                outr = outp.tile([128, D], F32, name="outr", tag="outr")
                for h in range(2):
                    hsl = slice(h * CW, (h + 1) * CW)
                    pa = ps_adj.tile([128, CW], F32, name="adj", tag="adj")
                    for r in range(4):
                        nc.tensor.matmul(
                            pa[:],
                            adjL[:, r, :, mt * 128 : (mt + 1) * 128],
                            proj[:, r * 2 : r * 2 + 2, hsl],
                            start=(r == 0), stop=(r == 3), perf_mode=DR,
                        )
                    nc.vector.scalar_tensor_tensor(
                        outt[:, hsl], pa[:], rd[:, mt : mt + 1], selfi[:, mt, hsl],
                        op0=ALU.mult, op1=ALU.add,
                    )
                nc.scalar.activation(outr[:], outt[:], AF.Relu)
                nc.sync.dma_start(out_d[b, mt * 128 : (mt + 1) * 128, :], outr[:])

        prev = None
        for b in range(NB):
            t = sweep(b)
            if prev is not None:
                finish(b - 1, prev)
            prev = t
        finish(NB - 1, prev)

    nc.compile()
    return nc


def _get_nc():
    if "nc" not in _cached:
        _cached["nc"] = _build_nc()
    return _cached["nc"]


def _dr_pack(mat):
    """[768, C] -> [128, 3, 2, C] with row k = kc*256 + j*128 + p."""
    c = mat.shape[1]
    return np.ascontiguousarray(mat.reshape(3, 2, 128, c).transpose(2, 0, 1, 3))


def _prep_in_maps(node, node_mask, argument_graph, punctuation_graph,
                  Wq, bq, Ws, bs, Wa, Wp, Wap, Wpp, Wa2, Wp2, Wap2, Wpp2):
    f32 = np.float32
    node = np.asarray(node, f32)
    mf = np.asarray(node_mask).astype(f32)
    A = np.asarray(argument_graph)
    P = np.asarray(punctuation_graph)

    Wac = np.asarray(Wa, f32) + np.asarray(Wa2, f32)
    Wpc = np.asarray(Wp, f32) + np.asarray(Wp2, f32)
    Wapc = np.asarray(Wap, f32) + np.asarray(Wap2, f32)
    Wppc = np.asarray(Wpp, f32) + np.asarray(Wpp2, f32)
    wbf = np.concatenate(
        [np.asarray(Ws, f32).T, np.asarray(Wq, f32).T], axis=1
    ).astype(BF)                                      # [768, 769]
    wrel = np.concatenate([Wac.T, Wpc.T, Wapc.T, Wppc.T], axis=1) * WSCALE
    w8 = _dr_pack(wrel.astype(F8))                    # [128, 3, 2, 3072]
    om = (1.0 - np.eye(N, dtype=f32)).astype(BF)
    bsb = np.broadcast_to(np.asarray(bs, f32), (128, D)).copy()
    bqb = np.full((128, 1), float(np.asarray(bq).reshape(-1)[0]), f32)

    nodeT_f = np.ascontiguousarray(node.transpose(0, 2, 1))      # [B, D, N]
    nodeT = nodeT_f.astype(BF)
    node8 = np.ascontiguousarray(
        nodeT_f.reshape(B, 3, 2, 128, N).transpose(0, 3, 1, 2, 4)
    ).astype(F8)                                      # [B, 128, 3, 2, N]
    ga = A.astype(BF)
    gat = np.ascontiguousarray(np.swapaxes(A, 1, 2)).astype(BF)
    gp = P.astype(BF)
    gpt = np.ascontiguousarray(np.swapaxes(P, 1, 2)).astype(BF)

    in_maps = []
    for c in range(NCORES):
        sl = slice(c * NB, (c + 1) * NB)
        in_maps.append(
            dict(
                nodeT=nodeT[sl], node8=node8[sl],
                ga=ga[sl], gat=gat[sl], gp=gp[sl], gpt=gpt[sl],
                mask=mf[sl], wbf=wbf, w8=w8, om=om, bsb=bsb, bqb=bqb,
            )
        )
    return in_maps


def _run(inputs, trace=False):
    from concourse.bass_utils import run_bass_kernel_spmd

    nc = _get_nc()
    in_maps = _prep_in_maps(**inputs)
    res = run_bass_kernel_spmd(
        nc, in_maps, core_ids=list(range(NCORES)), trace=trace
    )
    node_out = np.concatenate(
        [np.asarray(res.results[c]["out"], np.float32) for c in range(NCORES)], axis=0
    )
    aw = np.concatenate(
        [np.asarray(res.results[c]["aw"], np.float32) for c in range(NCORES)], axis=0
    )
    return (node_out, aw[:, None, :]), res


def kernel(**inputs):
    out, _ = _run(inputs, trace=False)
    return out


# revision 14
# speedup vs baseline: 1.5179x; 1.0490x over previous
"""Trainium2 Bass kernel for ArgumentGCN-with-reverse-edges (double relations).

Math (per batch b, N=256 nodes, D=768):
  dd     = mask outer product, zero diag (symmetric 0/1)
  g_a    = dd*A, g_p = dd*P, g_ar = dd*A^T, g_pr = dd*P^T   (dd idempotent)
  dw     = sigmoid(node @ Wq^T + bq)
  self   = node @ Ws^T + bs
  agg    = sum_x (dw-col-scaled g_x) @ (node @ Wxc^T),  Wxc = Wx + Wx2 (fused)
  out    = relu(self + agg / max(neigh,1)),  neigh = row-sums of the 4 graphs

Device mapping (per core: 8 batches, pure data parallel over 8 cores):
  - self_info + dw score: bf16 sweep  nodeT[768,256] x [Ws^T | Wq^T].
  - 4 relation projections: fp8(e4m3) DoubleRow matmuls — node and the
    combined relation weights (scaled by WSCALE to fit e4m3 range) packed
    as [128, kc, 2, *] K-pairs.  agg is only ~2% of output magnitude, so
    fp8 there is invisible at the output (verified ~3e-3 rel err).
  - Adjacency matmul K=1024 in fp8 DoubleRow: stacked dw-scaled graphs
    (no device transposes: host passes A and A^T; dd symmetry gives the
    rest) against the fp8 projections.
  - neigh via bf16 matmul with the mask vector (exact integer arithmetic).
  - Mask handled honestly: dwm = dw*m scales lhsT rows; the m_i row factor
    and the 1/WSCALE both fold into rdenom = m / (WSCALE * max(neigh,1)).
  - Two-stage software pipeline: PE does sweep(b) then adjacency(b-1), so
    drain/lhsT-construction latency never blocks the tensor engine.
"""

import os
import numpy as np
import ml_dtypes

BF = ml_dtypes.bfloat16
F8 = ml_dtypes.float8_e4m3

B, N, D = 64, 256, 768
NCORES = 8
NB = B // NCORES          # batches per core
CW = 384                  # psum chunk width
WSCALE = 32.0             # relation-weight scale for e4m3 range

_cached = {}


def _build_nc():
    import concourse.tile as tile
    from concourse import bacc, mybir
    from contextlib import ExitStack

    BF16 = mybir.dt.bfloat16
    FP8 = mybir.dt.float8e4
    F32 = mybir.dt.float32
    AF = mybir.ActivationFunctionType
    ALU = mybir.AluOpType
    DR = mybir.MatmulPerfMode.DoubleRow

    nc = bacc.Bacc(
        "TRN2",
        target_bir_lowering=False,
        debug=False,
        enable_asserts=False,
        num_devices=NCORES,
    )

    nodeT = nc.dram_tensor("nodeT", [NB, D, N], BF16, kind="ExternalInput")
    node8_d = nc.dram_tensor("node8", [NB, 128, 3, 2, N], FP8, kind="ExternalInput")
    gA_d = nc.dram_tensor("ga", [NB, N, N], BF16, kind="ExternalInput")
    gAT_d = nc.dram_tensor("gat", [NB, N, N], BF16, kind="ExternalInput")
    gP_d = nc.dram_tensor("gp", [NB, N, N], BF16, kind="ExternalInput")
    gPT_d = nc.dram_tensor("gpt", [NB, N, N], BF16, kind="ExternalInput")
    mask_d = nc.dram_tensor("mask", [NB, N], F32, kind="ExternalInput")
    wbf_d = nc.dram_tensor("wbf", [D, D + 1], BF16, kind="ExternalInput")
    w8_d = nc.dram_tensor("w8", [128, 3, 2, 4 * D], FP8, kind="ExternalInput")
    om_d = nc.dram_tensor("om", [N, N], BF16, kind="ExternalInput")
    bsb_d = nc.dram_tensor("bsb", [128, D], F32, kind="ExternalInput")
    bqb_d = nc.dram_tensor("bqb", [128, 1], F32, kind="ExternalInput")
    out_d = nc.dram_tensor("out", [NB, N, D], F32, kind="ExternalOutput")
    aw_d = nc.dram_tensor("aw", [NB, N], F32, kind="ExternalOutput")

    with tile.TileContext(nc) as tc, ExitStack() as ctx:
        const = ctx.enter_context(tc.tile_pool(name="const", bufs=1))
        inp = ctx.enter_context(tc.tile_pool(name="inp", bufs=3))
        work = ctx.enter_context(tc.tile_pool(name="work", bufs=2))
        outp = ctx.enter_context(tc.tile_pool(name="outp", bufs=3))
        ps_swp = ctx.enter_context(tc.tile_pool(name="ps_swp", bufs=5, space="PSUM"))
        ps_adj = ctx.enter_context(tc.tile_pool(name="ps_adj", bufs=3, space="PSUM"))

        wbf = const.tile([128, 6, D + 1], BF16, name="wbf")
        w8 = const.tile([128, 3, 2, 4 * D], FP8, name="w8")
        om = const.tile([128, 2, N], BF16, name="om")
        bsb = const.tile([128, D], F32, name="bsb")
        bqb = const.tile([128, 1], F32, name="bqb")
        wbf_r = wbf_d.rearrange("(c p) n -> p c n", p=128)

        def sweep(b):
            """DMA inputs; bf16 self/dw sweep; fp8 relation projections;
            adjacency-prep on DVE/GpSimd (overlaps next PE work)."""
            t = {}
            nT = inp.tile([128, 6, N], BF16, name="nT", tag="nT")
            nc.sync.dma_start(nT[:], nodeT[b].rearrange("(c p) n -> p c n", p=128))
            if b == 0:
                # startup-critical loads in consumption order on one queue
                nc.sync.dma_start(wbf[:, :, CW:], wbf_r[:, :, CW:])
                nc.sync.dma_start(wbf[:, :, :CW], wbf_r[:, :, :CW])
            n8 = inp.tile([128, 3, 2, N], FP8, name="n8", tag="n8")
            nc.sync.dma_start(n8[:], node8_d[b])
            if b == 0:
                for c in range(8):
                    csl = slice(c * CW, (c + 1) * CW)
                    nc.sync.dma_start(w8[:, :, :, csl], w8_d[:, :, :, csl])
            t["gA"] = gA = inp.tile([128, 2, N], BF16, name="gA", tag="gA")
            nc.sync.dma_start(gA[:], gA_d[b].rearrange("(t p) n -> p t n", p=128))
            t["gAT"] = gAT = inp.tile([128, 2, N], BF16, name="gAT", tag="gAT")
            nc.sync.dma_start(gAT[:], gAT_d[b].rearrange("(t p) n -> p t n", p=128))
            t["gP"] = gP = inp.tile([128, 2, N], BF16, name="gP", tag="gP")
            nc.sync.dma_start(gP[:], gP_d[b].rearrange("(t p) n -> p t n", p=128))
            t["gPT"] = gPT = inp.tile([128, 2, N], BF16, name="gPT", tag="gPT")
            nc.sync.dma_start(gPT[:], gPT_d[b].rearrange("(t p) n -> p t n", p=128))
            t["m32"] = m32 = inp.tile([128, 2], F32, name="m32", tag="m32")
            nc.sync.dma_start(m32[:], mask_d[b].rearrange("(t p) -> p t", p=128))
            if b == 0:
                nc.sync.dma_start(om[:], om_d.rearrange("(t p) n -> p t n", p=128))
                nc.sync.dma_start(bsb[:], bsb_d[:])
                nc.sync.dma_start(bqb[:], bqb_d[:])

            t["dw"] = dw = work.tile([128, 2], F32, name="dw", tag="dw")
            t["selfi"] = selfi = work.tile([128, 2, D], F32, name="selfi", tag="selfi")
            # col D holds v = m/dwm, the rhs column that turns the adjacency
            # matmul into the neigh row-sum (dw cancels against adjL's dwm)
            t["proj"] = proj = work.tile([128, 8, D + 2], FP8, name="proj", tag="proj")
            nc.vector.memset(proj[:, :, D + 1 : D + 2], 0.0)

            for mt in range(2):
                lhs = nT[:, :, mt * 128 : (mt + 1) * 128]
                # bf16: [Ws cols 384:768 | wq] first so dw is ready early
                ps1 = ps_swp.tile([128, 512], F32, name="swp", tag="swp")
                for k in range(6):
                    nc.tensor.matmul(
                        ps1[:, : CW + 1], lhs[:, k, :], wbf[:, k, CW:],
                        start=(k == 0), stop=(k == 5),
                    )
                nc.vector.tensor_add(selfi[:, mt, CW:], ps1[:, :CW], bsb[:, CW:])
                nc.scalar.activation(
                    dw[:, mt : mt + 1], ps1[:, CW : CW + 1], AF.Sigmoid, bias=bqb[:]
                )
                ps0 = ps_swp.tile([128, 512], F32, name="swp", tag="swp")
                for k in range(6):
                    nc.tensor.matmul(
                        ps0[:, :CW], lhs[:, k, :], wbf[:, k, :CW],
                        start=(k == 0), stop=(k == 5),
                    )
                nc.vector.tensor_add(selfi[:, mt, :CW], ps0[:, :CW], bsb[:, :CW])

                # fp8 DoubleRow relation projections
                lhs8 = n8[:, :, :, mt * 128 : (mt + 1) * 128]
                for c in range(8):
                    ps = ps_swp.tile([128, 512], F32, name="swp", tag="swp")
                    csl = slice(c * CW, (c + 1) * CW)
                    for kc in range(3):
                        nc.tensor.matmul(
                            ps[:, :CW], lhs8[:, kc, :, :], w8[:, kc, :, csl],
                            start=(kc == 0), stop=(kc == 2), perf_mode=DR,
                        )
                    r, h = divmod(c, 2)
                    dst = proj[:, r * 2 + mt, h * CW : (h + 1) * CW]
                    if c % 2 == 0:
                        nc.vector.tensor_copy(dst, ps[:, :CW])
                    else:
                        nc.scalar.activation(dst, ps[:, :CW], AF.Copy)

            nc.sync.dma_start(aw_d[b].rearrange("(t p) -> p t", p=128), dw[:])

            # ---- adjacency prep: DVE small ops + GpSimd bulk elementwise ----
            dwm = work.tile([128, 2], F32, name="dwm", tag="dwm")
            nc.vector.tensor_mul(dwm[:], dw[:], m32[:])
            ddw = work.tile([128, 2, N], BF16, name="ddw", tag="ddw")
            for tt in range(2):
                nc.vector.tensor_scalar_mul(ddw[:, tt, :], om[:, tt, :], dwm[:, tt : tt + 1])
            t["adjL"] = adjL = work.tile([128, 4, 2, N], FP8, name="adjL", tag="adjL")
            for r, g in enumerate([gAT, gPT, gA, gP]):
                nc.gpsimd.tensor_tensor(adjL[:, r, :, :], g[:], ddw[:], op=ALU.mult)
            # v column: clamp(1/max(dwm,eps), 400) * m, written per relation
            vt = work.tile([128, 2], F32, name="vt", tag="vt")
            nc.vector.tensor_scalar_max(vt[:], dwm[:], 1e-30)
            nc.vector.reciprocal(vt[:], vt[:])
            for r in range(4):
                nc.vector.scalar_tensor_tensor(
                    proj[:, 2 * r : 2 * r + 2, D : D + 1], vt[:], 400.0, m32[:],
                    op0=ALU.min, op1=ALU.mult,
                )
            return t

        def finish(b, t):
            """fp8 adjacency matmuls (neigh rides along as column D);
            rdenom; fused epilogue; out."""
            adjL, proj, selfi, m32 = t["adjL"], t["proj"], t["selfi"], t["m32"]
            for mt in range(2):
                msl = m32[:, mt : mt + 1]
                outt = outp.tile([128, D], F32, name="outt", tag="outt")
                outr = outp.tile([128, D], F32, name="outr", tag="outr")
                rd = work.tile([128, 1], F32, name="rd", tag="rd")
                for h in (1, 0):
                    w = CW + 2 if h == 1 else CW
                    hsl = slice(h * CW, h * CW + w)
                    pa = ps_adj.tile([128, CW + 2], F32, name="adj", tag="adj")
                    for r in range(4):
                        nc.tensor.matmul(
                            pa[:, :w],
                            adjL[:, r, :, mt * 128 : (mt + 1) * 128],
                            proj[:, r * 2 : r * 2 + 2, hsl],
                            start=(r == 0), stop=(r == 3), perf_mode=DR,
                        )
                    if h == 1:
                        # rd = m / (WSCALE * max(m*neigh, 1))
                        nc.vector.tensor_mul(rd[:], pa[:, CW : CW + 1], msl)
                        nc.vector.tensor_scalar_max(rd[:], rd[:], 1.0)
                        nc.vector.reciprocal(rd[:], rd[:])
                        nc.vector.scalar_tensor_tensor(
                            rd[:], rd[:], 1.0 / WSCALE, msl, op0=ALU.mult, op1=ALU.mult
                        )
                    nc.vector.scalar_tensor_tensor(
                        outt[:, h * CW : (h + 1) * CW], pa[:, :CW], rd[:],
                        selfi[:, mt, h * CW : (h + 1) * CW],
                        op0=ALU.mult, op1=ALU.add,
                    )
                nc.scalar.activation(outr[:], outt[:], AF.Relu)
                nc.sync.dma_start(out_d[b, mt * 128 : (mt + 1) * 128, :], outr[:])

        prev = None
        for b in range(NB):
            t = sweep(b)
            if prev is not None:
                finish(b - 1, prev)
            prev = t
        finish(NB - 1, prev)

    nc.compile()
    return nc


def _get_nc():
    if "nc" not in _cached:
        _cached["nc"] = _build_nc()
    return _cached["nc"]


def _dr_pack(mat):
    """[768, C] -> [128, 3, 2, C] with row k = kc*256 + j*128 + p."""
    c = mat.shape[1]
    return np.ascontiguousarray(mat.reshape(3, 2, 128, c).transpose(2, 0, 1, 3))


def _prep_in_maps(node, node_mask, argument_graph, punctuation_graph,
                  Wq, bq, Ws, bs, Wa, Wp, Wap, Wpp, Wa2, Wp2, Wap2, Wpp2):
    f32 = np.float32
    node = np.asarray(node, f32)
    mf = np.asarray(node_mask).astype(f32)
    A = np.asarray(argument_graph)
    P = np.asarray(punctuation_graph)

    Wac = np.asarray(Wa, f32) + np.asarray(Wa2, f32)
    Wpc = np.asarray(Wp, f32) + np.asarray(Wp2, f32)
    Wapc = np.asarray(Wap, f32) + np.asarray(Wap2, f32)
    Wppc = np.asarray(Wpp, f32) + np.asarray(Wpp2, f32)
    wbf = np.concatenate(
        [np.asarray(Ws, f32).T, np.asarray(Wq, f32).T], axis=1
    ).astype(BF)                                      # [768, 769]
    wrel = np.concatenate([Wac.T, Wpc.T, Wapc.T, Wppc.T], axis=1) * WSCALE
    w8 = _dr_pack(wrel.astype(F8))                    # [128, 3, 2, 3072]
    om = (1.0 - np.eye(N, dtype=f32)).astype(BF)
    bsb = np.broadcast_to(np.asarray(bs, f32), (128, D)).copy()
    bqb = np.full((128, 1), float(np.asarray(bq).reshape(-1)[0]), f32)

    nodeT_f = np.ascontiguousarray(node.transpose(0, 2, 1))      # [B, D, N]
    nodeT = nodeT_f.astype(BF)
    node8 = np.ascontiguousarray(
        nodeT_f.reshape(B, 3, 2, 128, N).transpose(0, 3, 1, 2, 4)
    ).astype(F8)                                      # [B, 128, 3, 2, N]
    ga = A.astype(BF)
    gat = np.ascontiguousarray(np.swapaxes(A, 1, 2)).astype(BF)
    gp = P.astype(BF)
    gpt = np.ascontiguousarray(np.swapaxes(P, 1, 2)).astype(BF)

    in_maps = []
    for c in range(NCORES):
        sl = slice(c * NB, (c + 1) * NB)
        in_maps.append(
            dict(
                nodeT=nodeT[sl], node8=node8[sl],
                ga=ga[sl], gat=gat[sl], gp=gp[sl], gpt=gpt[sl],
                mask=mf[sl], wbf=wbf, w8=w8, om=om, bsb=bsb, bqb=bqb,
            )
        )
    return in_maps


def _run(inputs, trace=False):
    from concourse.bass_utils import run_bass_kernel_spmd

    nc = _get_nc()
    in_maps = _prep_in_maps(**inputs)
    res = run_bass_kernel_spmd(
        nc, in_maps, core_ids=list(range(NCORES)), trace=trace
    )
    node_out = np.concatenate(
        [np.asarray(res.results[c]["out"], np.float32) for c in range(NCORES)], axis=0
    )
    aw = np.concatenate(
        [np.asarray(res.results[c]["aw"], np.float32) for c in range(NCORES)], axis=0
    )
    return (node_out, aw[:, None, :]), res


def kernel(**inputs):
    out, _ = _run(inputs, trace=False)
    return out
